# revision 5
# baseline (speedup 1.0000x reference)
"""Trainium2 Bass kernel for a 3-layer GCN + MLP scorer with neighbor-masked softmax.

The reference computes, for a graph with N nodes / E edges:
    h = tanh(GCN(tanh(GCN(tanh(GCN(x)))))); scores = MLP(h)
    out = softmax(scores masked to out-neighbors of current_vertex_idx)

The softmax mask makes the output exactly zero outside M = {out-neighbors of
cvi} | {cvi}.  Only the 3-hop *in*-neighborhood of M (a few hundred nodes of
the 50k) can influence the masked scores, so the kernel prunes the graph to
that closure on the host, builds small dense aggregation matrices (adjacency
with GCN normalization baked in), and runs the entire floating-point
computation on-device as a chain of dense matmuls + activations.  The device
program is SPMD-replicated across the 8 NeuronCores.

Host work is index-only (degree counts, BFS closure, building the per-call
aggregation matrices); every FLOP of the model runs on the NeuronCores.
"""

import numpy as np

D = 512      # node embedding size
H = 256      # predictor hidden size
F_IN = 16    # raw feature dim
ALPHA = 0.1  # leaky relu slope
N_CORES = 8
NEG = -1.0e30  # additive mask for padded softmax lanes

# Bucket caps: beyond these we fall back to the (identical-math) numpy path.
MAX_BUCKET = (4096, 1024, 256, 256)

_prog_cache: dict[tuple, object] = {}
last_results = None  # BassKernelResults of the most recent device run


# --------------------------------------------------------------------------
# Device program
# --------------------------------------------------------------------------

def _build_program(bucket):
    """Build the Bass/Tile program for padded sizes (n0, n1, n2, n3).

    Data layout convention (all f32):
      - "node-major" tiles t: [128 nodes, D feats] per tile  (matmul lhsT for
        aggregation steps, which contract over nodes)
      - "feature-major" tiles Hf: [128 feats, n nodes] per tile (matmul lhsT
        for the dense-layer steps, which contract over features)
    The chain t0 -> H1 -> t1 -> H2 -> t2 -> H3 -> P -> s alternates layouts so
    no on-device transposes are ever needed.
    """
    import concourse.bass as bass
    import concourse.bacc as bacc
    import concourse.mybir as mybir
    import concourse.tile as tile

    n0, n1, n2, n3 = bucket
    f32 = mybir.dt.float32
    ts = bass.ts
    KD = D // 128   # 4 feature tiles
    KH = H // 128   # 2 hidden tiles
    k0, k1, k2 = n0 // 128, n1 // 128, n2 // 128
    Tanh = mybir.ActivationFunctionType.Tanh
    Exp = mybir.ActivationFunctionType.Exp
    X = mybir.AxisListType.X

    nc = bacc.Bacc("TRN2", target_bir_lowering=False, debug=False)
    P_x0t = nc.declare_dram_parameter("x0t", [F_IN, n0], f32, isOutput=False)
    P_a1t = nc.declare_dram_parameter("a1t", [n0, n1], f32, isOutput=False)
    P_a2t = nc.declare_dram_parameter("a2t", [n1, n2], f32, isOutput=False)
    P_a3t = nc.declare_dram_parameter("a3t", [n2, n3], f32, isOutput=False)
    P_w1 = nc.declare_dram_parameter("w1", [F_IN, D], f32, isOutput=False)
    P_w2 = nc.declare_dram_parameter("w2", [D, D], f32, isOutput=False)
    P_w3 = nc.declare_dram_parameter("w3", [D, D], f32, isOutput=False)
    P_wp1 = nc.declare_dram_parameter("wp1", [D, H], f32, isOutput=False)
    P_wp2 = nc.declare_dram_parameter("wp2", [H, 1], f32, isOutput=False)
    P_b1 = nc.declare_dram_parameter("b1", [D, 1], f32, isOutput=False)
    P_b2 = nc.declare_dram_parameter("b2", [D, 1], f32, isOutput=False)
    P_b3 = nc.declare_dram_parameter("b3", [D, 1], f32, isOutput=False)
    P_bp1 = nc.declare_dram_parameter("bp1", [H, 1], f32, isOutput=False)
    P_mb = nc.declare_dram_parameter("maskb", [1, n3], f32, isOutput=False)
    P_out = nc.declare_dram_parameter("out", [1, n3], f32, isOutput=True)

    with tile.TileContext(nc) as tc:
        with (
            tc.tile_pool(name="sb", bufs=1) as sb,
            tc.tile_pool(name="ps", bufs=4, space="PSUM") as ps,
        ):
            def load(name, src, p, f):
                t = sb.tile([p, f], f32, tag=name)
                nc.sync.dma_start(t[:], src)
                return t

            x0t = load("x0t", P_x0t[:], F_IN, n0)
            w1 = load("w1", P_w1[:], F_IN, D)
            w2 = [load(f"w2_{i}", P_w2[ts(i, 128), :], 128, D) for i in range(KD)]
            w3 = [load(f"w3_{i}", P_w3[ts(i, 128), :], 128, D) for i in range(KD)]
            wp1 = [load(f"wp1_{i}", P_wp1[ts(i, 128), :], 128, H) for i in range(KD)]
            wp2 = [load(f"wp2_{i}", P_wp2[ts(i, 128), :], 128, 1) for i in range(KH)]
            b1 = [load(f"b1_{i}", P_b1[ts(i, 128), :], 128, 1) for i in range(KD)]
            b2 = [load(f"b2_{i}", P_b2[ts(i, 128), :], 128, 1) for i in range(KD)]
            b3 = [load(f"b3_{i}", P_b3[ts(i, 128), :], 128, 1) for i in range(KD)]
            bp1 = [load(f"bp1_{i}", P_bp1[ts(i, 128), :], 128, 1) for i in range(KH)]
            mb = load("maskb", P_mb[:], 1, n3)
            a1t = [load(f"a1t_{j}", P_a1t[ts(j, 128), :], 128, n1) for j in range(k0)]
            a2t = [load(f"a2t_{j}", P_a2t[ts(j, 128), :], 128, n2) for j in range(k1)]
            a3t = [load(f"a3t_{j}", P_a3t[ts(j, 128), :], 128, n3) for j in range(k2)]

            # t0[j] = x0[j-tile] @ W1   (node-major, K = F_IN)
            t0 = []
            for j in range(k0):
                acc = ps.tile([128, D], f32, tag="ps")
                nc.tensor.matmul(acc[:], x0t[:, ts(j, 128)], w1[:],
                                 start=True, stop=True)
                sbt = sb.tile([128, D], f32, tag=f"t0_{j}")
                nc.vector.tensor_copy(sbt[:], acc[:])
                t0.append(sbt)

            def agg_layer(t_in, k_in, aT, n_out, bias, lname):
                # Hf[di] = tanh( (A @ t).T tile + b )  : [128 feats, n_out]
                outs = []
                for di in range(KD):
                    ht = sb.tile([128, n_out], f32, tag=f"{lname}_{di}")
                    for c0 in range(0, n_out, 512):
                        cw = min(512, n_out - c0)
                        acc = ps.tile([128, cw], f32, tag="ps")
                        for j in range(k_in):
                            nc.tensor.matmul(
                                acc[:], t_in[j][:, ts(di, 128)],
                                aT[j][:, c0:c0 + cw],
                                start=(j == 0), stop=(j == k_in - 1))
                        nc.scalar.activation(ht[:, c0:c0 + cw], acc[:], Tanh,
                                             bias=bias[di][:])
                    outs.append(ht)
                return outs

            def dense_layer(Hf, n_rows, w_tiles, lname):
                # t[r] = h[r-tile] @ W : [128 nodes, D]
                outs = []
                for r in range(n_rows // 128):
                    acc = ps.tile([128, D], f32, tag="ps")
                    for di in range(KD):
                        nc.tensor.matmul(acc[:], Hf[di][:, ts(r, 128)],
                                         w_tiles[di][:],
                                         start=(di == 0), stop=(di == KD - 1))
                    sbt = sb.tile([128, D], f32, tag=f"{lname}_{r}")
                    nc.vector.tensor_copy(sbt[:], acc[:])
                    outs.append(sbt)
                return outs

            H1 = agg_layer(t0, k0, a1t, n1, b1, "h1")
            t1 = dense_layer(H1, n1, w2, "t1")
            H2 = agg_layer(t1, k1, a2t, n2, b2, "h2")
            t2 = dense_layer(H2, n2, w3, "t2")
            H3 = agg_layer(t2, k2, a3t, n3, b3, "h3")

            # predictor hidden: Pf[hi] = leaky_relu(Wp1.T @ h3.T + bp1)
            Pf = []
            for hi in range(KH):
                acc = ps.tile([128, n3], f32, tag="ps")
                for di in range(KD):
                    nc.tensor.matmul(acc[:], wp1[di][:, ts(hi, 128)], H3[di][:],
                                     start=(di == 0), stop=(di == KD - 1))
                z = sb.tile([128, n3], f32, tag=f"pz_{hi}")
                nc.vector.tensor_scalar_add(z[:], acc[:], bp1[hi][:])
                a = sb.tile([128, n3], f32, tag=f"pa_{hi}")
                nc.vector.tensor_scalar_mul(a[:], z[:], ALPHA)
                pm = sb.tile([128, n3], f32, tag=f"pm_{hi}")
                nc.vector.tensor_max(pm[:], z[:], a[:])
                Pf.append(pm)

            # scores s = Wp2.T @ P : [1, n3]; maskb = bp2 + (0 | -1e30)
            acc = ps.tile([1, n3], f32, tag="ps_s")
            for hi in range(KH):
                nc.tensor.matmul(acc[:], wp2[hi][:], Pf[hi][:],
                                 start=(hi == 0), stop=(hi == KH - 1))
            z = sb.tile([1, n3], f32, tag="sz")
            nc.vector.tensor_add(z[:], acc[:], mb[:])
            m = sb.tile([1, 1], f32, tag="sm")
            nc.vector.reduce_max(m[:], z[:], axis=X)
            negm = sb.tile([1, 1], f32, tag="negm")
            nc.vector.tensor_scalar_mul(negm[:], m[:], -1.0)
            e = sb.tile([1, n3], f32, tag="se")
            nc.scalar.activation(e[:], z[:], Exp, bias=negm[:])
            ssum = sb.tile([1, 1], f32, tag="ssum")
            nc.vector.reduce_sum(ssum[:], e[:], axis=X)
            rs = sb.tile([1, 1], f32, tag="rs")
            nc.vector.reciprocal(rs[:], ssum[:])
            o = sb.tile([1, n3], f32, tag="o")
            nc.vector.tensor_scalar_mul(o[:], e[:], rs[:])
            nc.sync.dma_start(P_out[:], o[:])

    nc.compile()
    return nc


def _get_program(bucket):
    prog = _prog_cache.get(bucket)
    if prog is None:
        prog = _build_program(bucket)
        _prog_cache[bucket] = prog
    return prog


# --------------------------------------------------------------------------
# Host-side graph pruning / packing
# --------------------------------------------------------------------------

def _next_size(n, minimum, step):
    n = max(n, minimum)
    r = minimum
    while r < n:
        r *= 2
    return ((r + step - 1) // step) * step


def _prune(vertices, src, dst, cvi):
    """Return (M, levels, edges, norms) for the 3-hop in-closure of M."""
    N = vertices.shape[0]
    indeg = np.bincount(dst, minlength=N)
    deg = (1.0 + indeg).astype(np.float32)
    dinv = (1.0 / np.sqrt(deg)).astype(np.float32)
    self_norm = (1.0 / deg).astype(np.float32)

    M = np.unique(np.concatenate([dst[src == cvi], [cvi]]))

    order = np.argsort(dst, kind="stable")
    dst_sorted = dst[order]
    src_sorted = src[order]
    rowptr = np.zeros(N + 1, dtype=np.int64)
    np.cumsum(np.bincount(dst_sorted, minlength=N), out=rowptr[1:])

    def in_edges_of(nodes):
        cs, cd = [], []
        for n in nodes:
            s, e = rowptr[n], rowptr[n + 1]
            cs.append(src_sorted[s:e])
            cd.append(dst_sorted[s:e])
        if cs:
            return np.concatenate(cs), np.concatenate(cd)
        z = np.array([], np.int64)
        return z, z

    L3 = M
    e3s, e3d = in_edges_of(L3)
    L2 = np.unique(np.concatenate([L3, e3s]))
    e2s, e2d = in_edges_of(L2)
    L1 = np.unique(np.concatenate([L2, e2s]))
    e1s, e1d = in_edges_of(L1)
    L0 = np.unique(np.concatenate([L1, e1s]))

    return (M, (L0, L1, L2, L3),
            ((e1s, e1d), (e2s, e2d), (e3s, e3d)), (dinv, self_norm))


def _build_aggT(rows_nodes, cols_nodes, es, ed, dinv, self_norm, nr, ncol):
    """A.T padded to [ncol, nr]: A[r,c] = sum(edge_norm) + self_norm on diag."""
    AT = np.zeros((ncol, nr), np.float32)
    r = np.searchsorted(rows_nodes, ed)
    c = np.searchsorted(cols_nodes, es)
    w = dinv[es] * dinv[ed]
    np.add.at(AT, (c, r), w)
    rr = np.arange(len(rows_nodes))
    cc = np.searchsorted(cols_nodes, rows_nodes)
    AT[cc, rr] += self_norm[rows_nodes]
    return AT


def _numpy_fallback(vertices, src, dst, cvi, W1, b1, W2, b2, W3, b3,
                    Wp1, bp1, Wp2, bp2):
    """Identical-math pruned computation in numpy (used only for graphs whose
    closure exceeds the device bucket caps)."""
    N = vertices.shape[0]
    M, levels, edges, (dinv, self_norm) = _prune(vertices, src, dst, cvi)
    L0, L1, L2, L3 = levels

    def agg(h, rows, cols, es, ed):
        loc_c = np.searchsorted(cols, es)
        loc_r = np.searchsorted(rows, ed)
        out = np.zeros((len(rows), h.shape[1]), np.float32)
        np.add.at(out, loc_r, h[loc_c] * (dinv[es] * dinv[ed])[:, None])
        out += h[np.searchsorted(cols, rows)] * self_norm[rows][:, None]
        return out

    (e1s, e1d), (e2s, e2d), (e3s, e3d) = edges
    t0 = vertices[L0].astype(np.float32) @ W1
    h1 = np.tanh(agg(t0, L1, L0, e1s, e1d) + b1)
    t1 = h1 @ W2
    h2 = np.tanh(agg(t1, L2, L1, e2s, e2d) + b2)
    t2 = h2 @ W3
    h3 = np.tanh(agg(t2, L3, L2, e3s, e3d) + b3)
    p = h3 @ Wp1 + bp1
    p = np.where(p >= 0, p, ALPHA * p)
    s = (p @ Wp2 + bp2)[:, 0]
    s = s - s.max()
    e = np.exp(s)
    out = np.zeros(N, np.float32)
    out[M] = e / e.sum()
    return out


# --------------------------------------------------------------------------
# Entry point
# --------------------------------------------------------------------------

def kernel(**inputs) -> np.ndarray:
    global last_results
    vertices = np.ascontiguousarray(np.asarray(inputs["vertices"], np.float32))
    edge_index = np.asarray(inputs["edge_index"])
    cvi = int(np.asarray(inputs["current_vertex_idx"]))
    W1 = np.ascontiguousarray(np.asarray(inputs["W1"], np.float32))
    W2 = np.ascontiguousarray(np.asarray(inputs["W2"], np.float32))
    W3 = np.ascontiguousarray(np.asarray(inputs["W3"], np.float32))
    Wp1 = np.ascontiguousarray(np.asarray(inputs["Wp1"], np.float32))
    Wp2 = np.ascontiguousarray(np.asarray(inputs["Wp2"], np.float32))
    b1 = np.asarray(inputs["b1"], np.float32)
    b2 = np.asarray(inputs["b2"], np.float32)
    b3 = np.asarray(inputs["b3"], np.float32)
    bp1 = np.asarray(inputs["bp1"], np.float32)
    bp2 = np.asarray(inputs["bp2"], np.float32)

    N = vertices.shape[0]
    src = np.asarray(edge_index[0], np.int64)
    dst = np.asarray(edge_index[1], np.int64)

    M, levels, edges, (dinv, self_norm) = _prune(vertices, src, dst, cvi)
    L0, L1, L2, L3 = levels
    (e1s, e1d), (e2s, e2d), (e3s, e3d) = edges

    n0 = _next_size(len(L0), 512, 128)
    n1 = _next_size(len(L1), 128, 128)
    n2 = _next_size(len(L2), 128, 128)
    n3 = _next_size(len(L3), 64, 64)
    bucket = (n0, n1, n2, n3)
    if any(b > cap for b, cap in zip(bucket, MAX_BUCKET)):
        return _numpy_fallback(vertices, src, dst, cvi, W1, b1, W2, b2,
                               W3, b3, Wp1, bp1, Wp2, bp2)

    x0t = np.zeros((F_IN, n0), np.float32)
    x0t[:, :len(L0)] = vertices[L0].T
    a1t = _build_aggT(L1, L0, e1s, e1d, dinv, self_norm, n1, n0)
    a2t = _build_aggT(L2, L1, e2s, e2d, dinv, self_norm, n2, n1)
    a3t = _build_aggT(L3, L2, e3s, e3d, dinv, self_norm, n3, n2)
    maskb = np.full((1, n3), NEG, np.float32)
    maskb[0, :len(M)] = float(bp2.reshape(-1)[0])

    in_map = {
        "x0t": x0t, "a1t": a1t, "a2t": a2t, "a3t": a3t,
        "w1": W1, "w2": W2, "w3": W3, "wp1": Wp1,
        "wp2": np.ascontiguousarray(Wp2.reshape(H, 1)),
        "b1": np.ascontiguousarray(b1.reshape(D, 1)),
        "b2": np.ascontiguousarray(b2.reshape(D, 1)),
        "b3": np.ascontiguousarray(b3.reshape(D, 1)),
        "bp1": np.ascontiguousarray(bp1.reshape(H, 1)),
        "maskb": maskb,
    }

    from concourse.bass_utils import run_bass_kernel_spmd
    nc = _get_program(bucket)
    last_results = run_bass_kernel_spmd(
        nc, [in_map] * N_CORES, list(range(N_CORES)))
    probs = np.asarray(last_results.results[0]["out"]).reshape(-1)

    out = np.zeros(N, np.float32)
    out[M] = probs[:len(M)]
    return out


# revision 9
# speedup vs baseline: 1.4060x; 1.4060x over previous
"""Trainium2 Bass kernel for a 3-layer GCN + MLP scorer with neighbor-masked softmax.

The reference computes, for a graph with N nodes / E edges:
    h = tanh(GCN(tanh(GCN(tanh(GCN(x)))))); scores = MLP(h)
    out = softmax(scores masked to out-neighbors of current_vertex_idx)

The softmax mask makes the output exactly zero outside M = {out-neighbors of
cvi} | {cvi}.  Only the 3-hop *in*-neighborhood of M (a few hundred nodes of
the 50k) can influence the masked scores, so the kernel prunes the graph to
that closure on the host, builds small dense aggregation matrices (adjacency
with GCN normalization baked in), and runs the entire floating-point
computation on-device as a chain of dense matmuls + activations.  The device
program is SPMD-replicated across the 8 NeuronCores.

Host work is index-only (degree counts, BFS closure, packing the per-call
aggregation matrices); every FLOP of the model runs on the NeuronCores.

Device-side notes:
  - Layer 1 is reassociated as (A1 @ x0) @ W1 (contract the 512-node dim
    first at F_IN=16 wide) instead of A1 @ (x0 @ W1) - about 4x fewer PE
    cycles for that layer.
  - Layouts alternate between node-major [nodes, D] and feature-major
    [D, nodes] so every matmul has its contraction dim on partitions and no
    on-device transposes are needed.
  - All inputs are host-packed into a handful of wide [128, F] blobs, loaded
    by parallel DMAs on different engine queues.
"""

import numpy as np

D = 512      # node embedding size
H = 256      # predictor hidden size
F_IN = 16    # raw node feature dim
ALPHA = 0.1  # leaky relu slope
N_CORES = 8
NEG = -1.0e30  # additive mask for padded softmax lanes

# Bucket caps: beyond these we fall back to the (identical-math) numpy path.
MAX_BUCKET = (4096, 512, 256, 256)

_prog_cache: dict[tuple, object] = {}
last_results = None  # BassKernelResults of the most recent device run


def _blob_layout(bucket):
    """Column layout of the packed [128, FB] input blob A."""
    n0, n1, n2, n3 = bucket
    k0, k1, k2 = n0 // 128, n1 // 128, n2 // 128
    off = 0
    lay = {}
    lay["x0"] = off; off += k0 * F_IN
    lay["a1"] = off; off += k0 * n1
    lay["a2"] = off; off += k1 * n2
    lay["a3"] = off; off += k2 * n3
    lay["b1"] = off; off += D // 128
    lay["b2"] = off; off += D // 128
    lay["b3"] = off; off += D // 128
    lay["bp1"] = off; off += H // 128
    lay["wp2"] = off; off += H // 128
    lay["_total"] = off
    return lay


# --------------------------------------------------------------------------
# Device program
# --------------------------------------------------------------------------

def _build_program(bucket):
    import concourse.bass as bass
    import concourse.bacc as bacc
    import concourse.mybir as mybir
    import concourse.tile as tile

    n0, n1, n2, n3 = bucket
    f32 = mybir.dt.float32
    ts = bass.ts
    KD = D // 128   # 4 feature tiles
    KH = H // 128   # 2 hidden tiles
    k0, k1, k2 = n0 // 128, n1 // 128, n2 // 128
    Tanh = mybir.ActivationFunctionType.Tanh
    Exp = mybir.ActivationFunctionType.Exp
    X = mybir.AxisListType.X
    lay = _blob_layout(bucket)
    FB = lay["_total"]

    nc = bacc.Bacc("TRN2", target_bir_lowering=False, debug=False)
    P_blob = nc.declare_dram_parameter("blob", [128, FB], f32, isOutput=False)
    P_w1 = nc.declare_dram_parameter("w1", [F_IN, D], f32, isOutput=False)
    P_w2 = nc.declare_dram_parameter("w2", [128, KD * D], f32, isOutput=False)
    P_w3 = nc.declare_dram_parameter("w3", [128, KD * D], f32, isOutput=False)
    P_wp1 = nc.declare_dram_parameter("wp1", [128, KD * H], f32, isOutput=False)
    P_mb = nc.declare_dram_parameter("maskb", [1, n3], f32, isOutput=False)
    P_out = nc.declare_dram_parameter("out", [1, n3], f32, isOutput=True)

    with tile.TileContext(nc) as tc:
        with (
            tc.tile_pool(name="sb", bufs=1) as sb,
            tc.tile_pool(name="ps", bufs=4, space="PSUM") as ps,
            tc.tile_pool(name="pss", bufs=2, space="PSUM") as pss,
        ):
            # parallel loads on separate engine queues; blob (needed first)
            # goes on the hardware DGE
            blob = sb.tile([128, FB], f32, tag="blob")
            nc.sync.dma_start(blob[:], P_blob[:])
            w1 = sb.tile([F_IN, D], f32, tag="w1")
            nc.sync.dma_start(w1[:], P_w1[:])
            mb = sb.tile([1, n3], f32, tag="maskb")
            nc.sync.dma_start(mb[:], P_mb[:])
            w2 = sb.tile([128, KD * D], f32, tag="w2")
            nc.gpsimd.dma_start(w2[:], P_w2[:])
            w3 = sb.tile([128, KD * D], f32, tag="w3")
            nc.scalar.dma_start(w3[:], P_w3[:])
            wp1 = sb.tile([128, KD * H], f32, tag="wp1")
            nc.gpsimd.dma_start(wp1[:], P_wp1[:])

            def bX(name, i):  # [128, 1] bias column for feature tile i
                return blob[:, lay[name] + i:lay[name] + i + 1]

            # ---- layer 1, reassociated: BT = (A1 @ x0).T = x0.T-contracted
            # BT[f, i] = sum_j x0[j, f] * A1T[j, i]   : [F_IN, n1]
            bt = sb.tile([F_IN, n1], f32, tag="bt")
            for c0 in range(0, n1, 512):
                cw = min(512, n1 - c0)
                acc = pss.tile([F_IN, cw], f32, tag="pss")
                for j in range(k0):
                    x0j = blob[:, lay["x0"] + j * F_IN:lay["x0"] + (j + 1) * F_IN]
                    a1j = blob[:, lay["a1"] + j * n1 + c0:lay["a1"] + j * n1 + c0 + cw]
                    nc.tensor.matmul(acc[:], x0j, a1j,
                                     start=(j == 0), stop=(j == k0 - 1))
                nc.vector.tensor_copy(bt[:, c0:c0 + cw], acc[:])

            # H1[di] = tanh(W1[:,dslice].T @ BT + b1)  : [128, n1] feature-major
            H1 = []
            for di in range(KD):
                ht = sb.tile([128, n1], f32, tag=f"h1_{di}")
                for c0 in range(0, n1, 512):
                    cw = min(512, n1 - c0)
                    acc = ps.tile([128, cw], f32, tag="ps")
                    nc.tensor.matmul(acc[:], w1[:, ts(di, 128)], bt[:, c0:c0 + cw],
                                     start=True, stop=True)
                    nc.scalar.activation(ht[:, c0:c0 + cw], acc[:], Tanh,
                                         bias=bX("b1", di))
                H1.append(ht)

            def dense_layer(Hf, n_rows, wt, lname):
                # t[r] = h[r-tile] @ W : node-major [128, D] tiles
                outs = []
                for r in range(n_rows // 128):
                    acc = ps.tile([128, D], f32, tag="ps")
                    for di in range(KD):
                        nc.tensor.matmul(acc[:], Hf[di][:, ts(r, 128)],
                                         wt[:, di * D:(di + 1) * D],
                                         start=(di == 0), stop=(di == KD - 1))
                    sbt = sb.tile([128, D], f32, tag=f"{lname}_{r}")
                    nc.vector.tensor_copy(sbt[:], acc[:])
                    outs.append(sbt)
                return outs

            def agg_layer(t_in, k_in, aname, n_out, bname, lname):
                # Hf[di] = tanh((A @ t).T tile + b) : [128, n_out] feature-major
                outs = []
                for di in range(KD):
                    ht = sb.tile([128, n_out], f32, tag=f"{lname}_{di}")
                    for c0 in range(0, n_out, 512):
                        cw = min(512, n_out - c0)
                        acc = ps.tile([128, cw], f32, tag="ps")
                        for j in range(k_in):
                            aj = blob[:, lay[aname] + j * n_out + c0:
                                      lay[aname] + j * n_out + c0 + cw]
                            nc.tensor.matmul(acc[:], t_in[j][:, ts(di, 128)], aj,
                                             start=(j == 0), stop=(j == k_in - 1))
                        nc.scalar.activation(ht[:, c0:c0 + cw], acc[:], Tanh,
                                             bias=bX(bname, di))
                    outs.append(ht)
                return outs

            t1 = dense_layer(H1, n1, w2, "t1")
            H2 = agg_layer(t1, k1, "a2", n2, "b2", "h2")
            t2 = dense_layer(H2, n2, w3, "t2")
            H3 = agg_layer(t2, k2, "a3", n3, "b3", "h3")

            # predictor hidden: Pf[hi] = leaky_relu(Wp1.T @ h3.T + bp1)
            Pf = []
            for hi in range(KH):
                acc = ps.tile([128, n3], f32, tag="ps")
                for di in range(KD):
                    nc.tensor.matmul(
                        acc[:], wp1[:, di * H + hi * 128:di * H + (hi + 1) * 128],
                        H3[di][:], start=(di == 0), stop=(di == KD - 1))
                z = sb.tile([128, n3], f32, tag=f"pz_{hi}")
                nc.vector.tensor_scalar_add(z[:], acc[:], bX("bp1", hi))
                a = sb.tile([128, n3], f32, tag=f"pa_{hi}")
                nc.vector.tensor_scalar_mul(a[:], z[:], ALPHA)
                pm = sb.tile([128, n3], f32, tag=f"pm_{hi}")
                nc.vector.tensor_max(pm[:], z[:], a[:])
                Pf.append(pm)

            # scores s = Wp2.T @ P : [1, n3]; maskb = bp2 + (0 | -1e30)
            acc = pss.tile([1, n3], f32, tag="pss")
            for hi in range(KH):
                nc.tensor.matmul(acc[:], bX("wp2", hi), Pf[hi][:],
                                 start=(hi == 0), stop=(hi == KH - 1))
            z = sb.tile([1, n3], f32, tag="sz")
            nc.vector.tensor_add(z[:], acc[:], mb[:])
            m = sb.tile([1, 1], f32, tag="sm")
            nc.vector.reduce_max(m[:], z[:], axis=X)
            negm = sb.tile([1, 1], f32, tag="negm")
            nc.vector.tensor_scalar_mul(negm[:], m[:], -1.0)
            e = sb.tile([1, n3], f32, tag="se")
            nc.scalar.activation(e[:], z[:], Exp, bias=negm[:])
            ssum = sb.tile([1, 1], f32, tag="ssum")
            nc.vector.reduce_sum(ssum[:], e[:], axis=X)
            rs = sb.tile([1, 1], f32, tag="rs")
            nc.vector.reciprocal(rs[:], ssum[:])
            o = sb.tile([1, n3], f32, tag="o")
            nc.vector.tensor_scalar_mul(o[:], e[:], rs[:])
            nc.sync.dma_start(P_out[:], o[:])

    nc.compile()
    return nc


def _get_program(bucket):
    prog = _prog_cache.get(bucket)
    if prog is None:
        prog = _build_program(bucket)
        _prog_cache[bucket] = prog
    return prog


# --------------------------------------------------------------------------
# Host-side graph pruning / packing
# --------------------------------------------------------------------------

def _next_size(n, minimum):
    r = minimum
    while r < n:
        r *= 2
    return r


def _prune(N, src, dst, cvi):
    """Return (M, levels, edges, norms) for the 3-hop in-closure of M."""
    indeg = np.bincount(dst, minlength=N)
    deg = (1.0 + indeg).astype(np.float32)
    dinv = (1.0 / np.sqrt(deg)).astype(np.float32)
    self_norm = (1.0 / deg).astype(np.float32)

    M = np.unique(np.concatenate([dst[src == cvi], [cvi]]))

    order = np.argsort(dst, kind="stable")
    dst_sorted = dst[order]
    src_sorted = src[order]
    rowptr = np.zeros(N + 1, dtype=np.int64)
    np.cumsum(np.bincount(dst_sorted, minlength=N), out=rowptr[1:])

    def in_edges_of(nodes):
        cs, cd = [], []
        for n in nodes:
            s, e = rowptr[n], rowptr[n + 1]
            cs.append(src_sorted[s:e])
            cd.append(dst_sorted[s:e])
        if cs:
            return np.concatenate(cs), np.concatenate(cd)
        z = np.array([], np.int64)
        return z, z

    L3 = M
    e3s, e3d = in_edges_of(L3)
    L2 = np.unique(np.concatenate([L3, e3s]))
    e2s, e2d = in_edges_of(L2)
    L1 = np.unique(np.concatenate([L2, e2s]))
    e1s, e1d = in_edges_of(L1)
    L0 = np.unique(np.concatenate([L1, e1s]))

    return (M, (L0, L1, L2, L3),
            ((e1s, e1d), (e2s, e2d), (e3s, e3d)), (dinv, self_norm))


def _build_aggT(rows_nodes, cols_nodes, es, ed, dinv, self_norm, nr, ncol):
    """A.T zero-padded to [ncol, nr]: A[r,c] = sum(edge_norm) + self_norm diag."""
    AT = np.zeros((ncol, nr), np.float32)
    r = np.searchsorted(rows_nodes, ed)
    c = np.searchsorted(cols_nodes, es)
    w = dinv[es] * dinv[ed]
    np.add.at(AT, (c, r), w)
    rr = np.arange(len(rows_nodes))
    cc = np.searchsorted(cols_nodes, rows_nodes)
    AT[cc, rr] += self_norm[rows_nodes]
    return AT


def _tile128(a2d, k):
    """[k*128, f] -> [128, k*f] with tile j at columns [j*f, (j+1)*f)."""
    f = a2d.shape[1]
    return np.ascontiguousarray(
        a2d.reshape(k, 128, f).transpose(1, 0, 2).reshape(128, k * f))


def _numpy_fallback(vertices, src, dst, cvi, W1, b1, W2, b2, W3, b3,
                    Wp1, bp1, Wp2, bp2):
    """Identical-math pruned computation in numpy (used only for graphs whose
    closure exceeds the device bucket caps)."""
    N = vertices.shape[0]
    M, levels, edges, (dinv, self_norm) = _prune(N, src, dst, cvi)
    L0, L1, L2, L3 = levels

    def agg(h, rows, cols, es, ed):
        loc_c = np.searchsorted(cols, es)
        loc_r = np.searchsorted(rows, ed)
        out = np.zeros((len(rows), h.shape[1]), np.float32)
        np.add.at(out, loc_r, h[loc_c] * (dinv[es] * dinv[ed])[:, None])
        out += h[np.searchsorted(cols, rows)] * self_norm[rows][:, None]
        return out

    (e1s, e1d), (e2s, e2d), (e3s, e3d) = edges
    t0 = vertices[L0].astype(np.float32) @ W1
    h1 = np.tanh(agg(t0, L1, L0, e1s, e1d) + b1)
    t1 = h1 @ W2
    h2 = np.tanh(agg(t1, L2, L1, e2s, e2d) + b2)
    t2 = h2 @ W3
    h3 = np.tanh(agg(t2, L3, L2, e3s, e3d) + b3)
    p = h3 @ Wp1 + bp1
    p = np.where(p >= 0, p, ALPHA * p)
    s = (p @ Wp2 + bp2)[:, 0]
    s = s - s.max()
    e = np.exp(s)
    out = np.zeros(N, np.float32)
    out[M] = e / e.sum()
    return out


# --------------------------------------------------------------------------
# Entry point
# --------------------------------------------------------------------------

def kernel(**inputs) -> np.ndarray:
    global last_results
    vertices = np.ascontiguousarray(np.asarray(inputs["vertices"], np.float32))
    edge_index = np.asarray(inputs["edge_index"])
    cvi = int(np.asarray(inputs["current_vertex_idx"]))
    W1 = np.asarray(inputs["W1"], np.float32)
    W2 = np.asarray(inputs["W2"], np.float32)
    W3 = np.asarray(inputs["W3"], np.float32)
    Wp1 = np.asarray(inputs["Wp1"], np.float32)
    Wp2 = np.asarray(inputs["Wp2"], np.float32)
    b1 = np.asarray(inputs["b1"], np.float32)
    b2 = np.asarray(inputs["b2"], np.float32)
    b3 = np.asarray(inputs["b3"], np.float32)
    bp1 = np.asarray(inputs["bp1"], np.float32)
    bp2 = np.asarray(inputs["bp2"], np.float32)

    N = vertices.shape[0]
    src = np.asarray(edge_index[0], np.int64)
    dst = np.asarray(edge_index[1], np.int64)

    M, levels, edges, (dinv, self_norm) = _prune(N, src, dst, cvi)
    L0, L1, L2, L3 = levels
    (e1s, e1d), (e2s, e2d), (e3s, e3d) = edges

    n0 = _next_size(len(L0), 256)
    n1 = _next_size(len(L1), 128)
    n2 = _next_size(len(L2), 128)
    n3 = _next_size(len(L3), 64)
    bucket = (n0, n1, n2, n3)
    if any(b > cap for b, cap in zip(bucket, MAX_BUCKET)):
        return _numpy_fallback(vertices, src, dst, cvi, W1, b1, W2, b2,
                               W3, b3, Wp1, bp1, Wp2, bp2)
    k0, k1, k2 = n0 // 128, n1 // 128, n2 // 128

    x0 = np.zeros((n0, F_IN), np.float32)
    x0[:len(L0)] = vertices[L0]
    a1t = _build_aggT(L1, L0, e1s, e1d, dinv, self_norm, n1, n0)
    a2t = _build_aggT(L2, L1, e2s, e2d, dinv, self_norm, n2, n1)
    a3t = _build_aggT(L3, L2, e3s, e3d, dinv, self_norm, n3, n2)
    maskb = np.full((1, n3), NEG, np.float32)
    maskb[0, :len(M)] = float(bp2.reshape(-1)[0])

    blob = np.concatenate([
        _tile128(x0, k0),
        _tile128(a1t, k0),
        _tile128(a2t, k1),
        _tile128(a3t, k2),
        b1.reshape(KD := D // 128, 128).T,
        b2.reshape(KD, 128).T,
        b3.reshape(KD, 128).T,
        bp1.reshape(H // 128, 128).T,
        Wp2.reshape(H // 128, 128).T,
    ], axis=1)
    blob = np.ascontiguousarray(blob, np.float32)
    assert blob.shape[1] == _blob_layout(bucket)["_total"]

    in_map = {
        "blob": blob,
        "w1": np.ascontiguousarray(W1),
        "w2": _tile128(W2, KD),
        "w3": _tile128(W3, KD),
        "wp1": _tile128(Wp1, KD),
        "maskb": maskb,
    }

    from concourse.bass_utils import run_bass_kernel_spmd
    nc = _get_program(bucket)
    last_results = run_bass_kernel_spmd(
        nc, [in_map] * N_CORES, list(range(N_CORES)))
    probs = np.asarray(last_results.results[0]["out"]).reshape(-1)

    out = np.zeros(N, np.float32)
    out[M] = probs[:len(M)]
    return out
